# revision 1
# baseline (speedup 1.0000x reference)
"""GAT-style attention kernel for Trainium2 (8 NeuronCores, row-parallel).

Computation (per reference):
    scores    = tanh(einsum('ijk,ko->ijo', edges, W))        # (N, N, 1)
    attention = softmax(scores, axis=1).reshape(N, N)        # row softmax over j
    out       = tanh(attention @ features)                   # (N, D_FEAT)

Sharding: rows (i axis) split across the 8 cores; features/W replicated; no
cross-core communication (each row's softmax + aggregation is local).

Per-core pipeline, fully pipelined per j-tile (i-blocks of 128 rows on the
partition axis; j tiled by `jt`):
  1. DMA  edges tile (128 i, jt j, 16 k) -- per-partition contiguous 32KB
  2. DVE  scores_slice = sum_k E[:,:,k]*W[k] as a chain of 16 in-place
          scalar_tensor_tensor ops (scalar = per-partition W[k] broadcast);
          ONE 1x-rate pass over the data -- this is the key trick that makes
          the kernel DMA-bound instead of DVE-bound (fp32 tensor_tensor +
          tensor_reduce would be two passes).
  3. ACT  tanh then exp in place on the slice; exp's accum_out collects the
          per-slice row-sum partials of Z.
  4. PE   transpose each 128x128 att chunk (SBUF->PSUM via identity),
          ScalarE copies PSUM->SBUF, and PE immediately accumulates
          attT_chunk.T @ features_chunk into the PSUM output tile.
  5. DVE  Z = sum of partials; rz = 1/Z.
  6. ACT  out = tanh(psum * rz) (per-partition scale), DMA out.

Numerics: softmax skips the max-subtraction (scores are tanh-bounded in
(-1,1) so exp cannot overflow) and normalization is folded to the end
(aggregation is linear in att). All arithmetic fp32; HW rel err vs the jax
reference ~2.6e-6.

Roofline: the 1.07GB edges tensor must stream from HBM once; 134MB/core at
~360GB/s -> ~373us of DMA busy. Scores live in small per-tile buffers
(not a full [128, N] row), freeing SBUF for edge-tile buffers.

jt=1024 (measured A/B on HW): the observed DVE cost per chain op carries a
large fixed per-instruction overhead (~0.4us) on this hardware, so wider
tiles -- which halve the DVE/ACT/DMA instruction counts per element --
measured ~10% faster per rep than jt=512 despite the shallower (2-deep)
edge-buffer pipeline.
"""

from contextlib import ExitStack

import numpy as np

import concourse.bass as bass
import concourse.bacc as bacc
import concourse.tile as tile
from concourse import mybir
from concourse.bass_utils import run_bass_kernel_spmd
from concourse.masks import make_identity

F32 = mybir.dt.float32
AF = mybir.ActivationFunctionType
ALU = mybir.AluOpType
AX = mybir.AxisListType

N_CORES = 8


def _schedule(n, jt, kind):
    """Per-block j-tile size lists (ramp for block 0, taper for the last)."""
    base = [jt] * (n // jt)
    if kind == "uniform" or jt != 512 or n < 2048:
        return base, base, base
    if kind == "ramp":
        ramp = [128, 128, 256] + [jt] * ((n - 512) // jt)
        taper = [jt] * ((n - 1024) // jt) + [512, 256, 256]
    elif kind == "ramp256":
        ramp = [256, 256] + [jt] * ((n - 512) // jt)
        taper = [jt] * ((n - 512) // jt) + [256, 256]
    elif kind == "tail128":
        ramp = [256, 256] + [jt] * ((n - 512) // jt)
        taper = [jt] * ((n - 512) // jt) + [256, 128, 128]
    else:
        raise ValueError(kind)
    assert sum(ramp) == n and sum(taper) == n
    return ramp, base, taper


def build(n=4096, de=16, df=128, n_cores=N_CORES, jt=1024, reps=1,
          ebufs=None, sbufs=4, sched="ramp256", gp_k=0, dummy="none",
          mode="chain", copy_eng="scalar"):
    """Build the per-core Bass program. Returns nc.

    gp_k: number of trailing k-steps of the score chain offloaded to the
    GPSIMD (Pool) engine into a separate partial, combined by one DVE add.

    dummy: benchmarking diagnostic only (kernel() always uses "none").
    For reps > 1, all but the LAST rep are built with identical instruction
    structure/count but altered access-pattern sizes, so per-call host
    overhead (which scales with program size) cancels in a reps-slope while
    device-exec content changes:
      "compute": DVE/ACT ops shrunk to 16-wide, edge DMAs kept FULL size
                 -> extra reps cost ~ the pure DMA stream.
      "tiny":    edge DMAs shrunk to 16-wide too -> extra reps cost ~
                 instruction-issue only.
    The final rep is always the real computation, so outputs stay correct.
    """
    if ebufs is None:
        # as many edge-tile buffers as fit beside ~30KB/partition of other
        # tiles (jt=1024 -> 2, jt=512 -> 5)
        ebufs = max(2, (208 - 30) * 1024 // (jt * de * 4))
    rows = n // n_cores          # i-rows per core
    iblk = 128                   # i-rows per block (partition dim)
    nblk = rows // iblk
    nck = n // 128               # 128-wide j chunks (for transpose/aggregation)
    if mode == "ttr":
        ebufs = min(ebufs, 4)    # make room for the ttr scratch tile

    nc = bacc.Bacc("TRN2", target_bir_lowering=False, debug=False)
    ed = nc.dram_tensor("edges", [rows, n, de], F32, kind="ExternalInput")
    ft = nc.dram_tensor("features", [n, df], F32, kind="ExternalInput")
    wd = nc.dram_tensor("W", [de, 1], F32, kind="ExternalInput")
    out = nc.dram_tensor("out", [rows, df], F32, kind="ExternalOutput")

    with tile.TileContext(nc) as tc, ExitStack() as ctx:
        consts = ctx.enter_context(tc.tile_pool(name="consts", bufs=1))
        epool = ctx.enter_context(tc.tile_pool(name="epool", bufs=ebufs))
        spool = ctx.enter_context(tc.tile_pool(name="spool", bufs=sbufs))
        if gp_k:
            gpool = ctx.enter_context(tc.tile_pool(name="gpool", bufs=sbufs))
        if mode == "ttr":
            xpool = ctx.enter_context(tc.tile_pool(name="xpool", bufs=1))
        if mode == "ilv2":
            i2pool = ctx.enter_context(tc.tile_pool(name="i2pool", bufs=2))
        tpool = ctx.enter_context(tc.tile_pool(name="tpool", bufs=4))
        mpool = ctx.enter_context(tc.tile_pool(name="mpool", bufs=4))
        opool = ctx.enter_context(tc.tile_pool(name="opool", bufs=2))
        ppool = ctx.enter_context(tc.tile_pool(name="ppool", bufs=4, space="PSUM"))
        upool = ctx.enter_context(tc.tile_pool(name="upool", bufs=2, space="PSUM"))

        # --- constants -----------------------------------------------------
        ramp, base, taper = _schedule(n, jt, sched)
        dve_k = de - gp_k

        ident = consts.tile([128, 128], F32)
        make_identity(nc, ident)

        # features chunks: featx[p, c, d] = features[128c + p, d]
        featx = consts.tile([128, nck, df], F32)
        nc.sync.dma_start(
            out=featx[:], in_=ft.rearrange("(c p) d -> p c d", p=128)
        )

        # W broadcast to all partitions: wall[p, k] = W[k]
        # (copied through DVE so the hot TT mul only waits on the edges DMA —
        # the 3-operand TT ISA encoding has a single sync-wait slot)
        wall_raw = consts.tile([128, de], F32)
        nc.sync.dma_start(out=wall_raw[:], in_=bass.AP(wd, 0, [[0, 128], [1, de]]))
        wall = consts.tile([128, de], F32)
        nc.vector.tensor_copy(wall[:], wall_raw[:])
        if gp_k:
            wall_gp = consts.tile([128, de], F32)
            nc.vector.tensor_copy(wall_gp[:], wall_raw[:])
        if mode == "ttr":
            # scratch for tensor_tensor_reduce's product output. bufs=1 is
            # safe: nothing reads it, and the WAW dependency just serializes
            # the ttr ops, which run serially on DVE anyway.
            xscr = xpool.tile([128, jt, de], F32)

        # --- main loop -----------------------------------------------------
        for _rep in range(reps):
          is_dummy = dummy != "none" and _rep != reps - 1
          for b in range(nblk):
              sched_b = ramp if b == 0 else (taper if b == nblk - 1 else base)
              nzp = len(sched_b)
              zparts = mpool.tile([128, nzp], F32, tag="zparts")
              pu = upool.tile([128, df], F32)
              j0 = 0
              for q, sz in enumerate(sched_b):
                  csz = 16 if is_dummy else sz          # compute width
                  dsz = (16 if dummy == "tiny" else sz) if is_dummy else sz
                  et = epool.tile([128, jt, de], F32)
                  nc.sync.dma_start(
                      out=et[:, 0:dsz, :],
                      in_=ed[b * iblk:(b + 1) * iblk, j0:j0 + dsz, :],
                  )
                  # scores_slice = sum_k E[:, :, k] * W[k]
                  stile = spool.tile([128, jt], F32, tag="stile")
                  ssl = stile[:, 0:csz]
                  if mode == "ttr":
                      # ONE fused DVE op per tile: product with broadcast W
                      # (contiguous reads) + grouped row-sum over k into the
                      # scores slice. 16x fewer DVE instructions than the
                      # scalar_tensor_tensor chain and no strided reads.
                      wbc = wall.rearrange("p k -> p () k").broadcast_to(
                          [128, csz, de])
                      nc.vector.tensor_tensor_reduce(
                          out=xscr[:, 0:csz, :], in0=et[:, 0:csz, :], in1=wbc,
                          scale=1.0, scalar=0.0, op0=ALU.mult, op1=ALU.add,
                          accum_out=stile.rearrange("p j -> p j ()")[:, 0:csz, :],
                      )
                  elif mode == "ilv2":
                      # two independent interleaved partial chains: even k
                      # accumulate into ssl, odd k into p2sl.  Consecutive
                      # DVE instructions then never have a RAW dependency,
                      # so the engine pipeline never drains mid-chain.
                      # One extra combine add at the end.
                      p2 = i2pool.tile([128, jt], F32, tag="p2")
                      p2sl = p2[:, 0:csz]
                      nc.vector.tensor_scalar(
                          ssl, et[:, 0:csz, 0], wall[:, 0:1], None, ALU.mult)
                      nc.vector.tensor_scalar(
                          p2sl, et[:, 0:csz, 1], wall[:, 1:2], None, ALU.mult)
                      for k in range(2, dve_k):
                          dst = ssl if k % 2 == 0 else p2sl
                          nc.vector.scalar_tensor_tensor(
                              dst, et[:, 0:csz, k], wall[:, k:k + 1], dst,
                              ALU.mult, ALU.add,
                          )
                      nc.vector.tensor_tensor(ssl, p2sl, ssl, ALU.add)
                  else:
                      # chain of 16 in-place scalar_tensor_tensor ops
                      # (scalar = per-partition W[k] broadcast)
                      nc.vector.tensor_scalar(
                          ssl, et[:, 0:csz, 0], wall[:, 0:1], None, ALU.mult)
                      for k in range(1, dve_k):
                          nc.vector.scalar_tensor_tensor(
                              ssl, et[:, 0:csz, k], wall[:, k:k + 1], ssl,
                              ALU.mult, ALU.add,
                          )
                  if gp_k:
                      gtile = gpool.tile([128, jt], F32, tag="gtile")
                      gsl = gtile[:, 0:csz]
                      nc.gpsimd.tensor_scalar(
                          gsl, et[:, 0:csz, dve_k], wall_gp[:, dve_k:dve_k + 1],
                          None, ALU.mult)
                      for k in range(dve_k + 1, de):
                          nc.gpsimd.scalar_tensor_tensor(
                              gsl, et[:, 0:csz, k], wall_gp[:, k:k + 1], gsl,
                              ALU.mult, ALU.add,
                          )
                      nc.vector.tensor_tensor(ssl, gsl, ssl, ALU.add)
                  # per-slice softmax numerator + partial row-sum
                  nc.scalar.activation(ssl, ssl, AF.Tanh)
                  nc.scalar.activation(ssl, ssl, AF.Exp,
                                       accum_out=zparts[:, q:q + 1])
                  # transpose + aggregate this slice's 128-chunks immediately
                  for ci in range(sz // 128):
                      c = j0 // 128 + ci
                      ptile = ppool.tile([128, 128], F32)
                      nc.tensor.transpose(
                          ptile[:], stile[:, ci * 128:(ci + 1) * 128], ident[:]
                      )
                      atc = tpool.tile([128, 128], F32)
                      if copy_eng == "gpsimd":
                          # PSUM->SBUF staging on the otherwise-idle Pool
                          # engine, keeping ACT free for tanh/exp
                          nc.gpsimd.tensor_copy(atc[:], ptile[:])
                      else:
                          nc.scalar.copy(atc[:], ptile[:])
                      nc.tensor.matmul(
                          pu[:], atc[:], featx[:, c, :],
                          start=(c == 0), stop=(c == nck - 1),
                      )
                  j0 += sz

              zcol = mpool.tile([128, 1], F32)
              nc.vector.reduce_sum(out=zcol[:], in_=zparts[:], axis=AX.X)
              rz = mpool.tile([128, 1], F32)
              nc.vector.reciprocal(rz[:], zcol[:])

              # out = tanh(pu * rz)
              osb = opool.tile([128, df], F32)
              nc.scalar.activation(osb[:], pu[:], AF.Tanh, scale=rz[:])
              # out-DMA on the scalar-engine HWDGE ring: edge DMAs (SP ring)
              # never queue behind it (HWDGE is FIFO per issuing engine)
              nc.scalar.dma_start(
                  out=out[b * iblk:(b + 1) * iblk, :], in_=osb[:]
              )

    nc.compile()
    return nc


_CACHE = {}


def _get_nc(shape_key):
    if shape_key not in _CACHE:
        n, de, df = shape_key
        _CACHE[shape_key] = build(n=n, de=de, df=df)
    return _CACHE[shape_key]


def kernel(edges, features, W):
    n, n2, de = edges.shape
    df = features.shape[1]
    rows = n // N_CORES
    nc = _get_nc((n, de, df))

    edges = np.ascontiguousarray(edges, dtype=np.float32)
    features = np.ascontiguousarray(features, dtype=np.float32)
    W = np.ascontiguousarray(W, dtype=np.float32)

    in_maps = [
        {
            "edges": edges[c * rows:(c + 1) * rows],
            "features": features,
            "W": W,
        }
        for c in range(N_CORES)
    ]
    res = run_bass_kernel_spmd(nc, in_maps, core_ids=list(range(N_CORES)))
    return np.concatenate([r["out"] for r in res.results], axis=0)



# revision 11
# speedup vs baseline: 1.3066x; 1.3066x over previous
"""GAT-style attention kernel for Trainium2 (8 NeuronCores, row-parallel).

Computation (per reference):
    scores    = tanh(einsum('ijk,ko->ijo', edges, W))        # (N, N, 1)
    attention = softmax(scores, axis=1).reshape(N, N)        # row softmax over j
    out       = tanh(attention @ features)                   # (N, D_FEAT)

Sharding: rows (i axis) split across the 8 cores; features/W replicated; no
cross-core communication (each row's softmax + aggregation is local).

Per-core pipeline, fully pipelined per j-tile (i-blocks of 128 rows on the
partition axis; j tiled by `jt`):
  1. DMA  edges tile (128 i, jt j, 16 k) -- per-partition contiguous 32KB
  2. DVE  scores_slice = sum_k E[:,:,k]*W[k] as a chain of 16 in-place
          scalar_tensor_tensor ops (scalar = per-partition W[k] broadcast);
          ONE 1x-rate pass over the data -- this is the key trick that makes
          the kernel DMA-bound instead of DVE-bound (fp32 tensor_tensor +
          tensor_reduce would be two passes).
  3. ACT  tanh then exp in place on the slice; exp's accum_out collects the
          per-slice row-sum partials of Z.
  4. PE   transpose each 128x128 att chunk (SBUF->PSUM via identity),
          ScalarE copies PSUM->SBUF, and PE immediately accumulates
          attT_chunk.T @ features_chunk into the PSUM output tile.
  5. DVE  Z = sum of partials; rz = 1/Z.
  6. ACT  out = tanh(psum * rz) (per-partition scale), DMA out.

Numerics: softmax skips the max-subtraction (scores are tanh-bounded in
(-1,1) so exp cannot overflow) and normalization is folded to the end
(aggregation is linear in att). All arithmetic fp32; HW rel err vs the jax
reference ~2.6e-6.

Roofline: the 1.07GB edges tensor must stream from HBM once; 134MB/core at
~360GB/s -> ~373us of DMA busy. Scores live in small per-tile buffers
(not a full [128, N] row), freeing SBUF for edge-tile buffers.

jt=1024 (measured A/B on HW): the observed DVE cost per chain op carries a
large fixed per-instruction overhead (~0.4us) on this hardware, so wider
tiles -- which halve the DVE/ACT/DMA instruction counts per element --
measured ~10% faster per rep than jt=512 despite the shallower (2-deep)
edge-buffer pipeline.
"""

from contextlib import ExitStack

import numpy as np

import concourse.bass as bass
import concourse.bacc as bacc
import concourse.tile as tile
from concourse import mybir
from concourse.bass_utils import run_bass_kernel_spmd
from concourse.masks import make_identity

F32 = mybir.dt.float32
AF = mybir.ActivationFunctionType
ALU = mybir.AluOpType
AX = mybir.AxisListType

N_CORES = 8


def _schedule(n, jt, kind):
    """Per-block j-tile size lists (ramp for block 0, taper for the last)."""
    base = [jt] * (n // jt)
    if kind == "tail1024" and jt == 1024 and n >= 4096:
        # taper the final block so the post-DMA compute drain is short
        taper = [jt] * ((n - 1024) // jt) + [512, 256, 128, 128]
        assert sum(taper) == n
        return base, base, taper
    if kind == "uniform" or jt != 512 or n < 2048:
        return base, base, base
    if kind == "ramp":
        ramp = [128, 128, 256] + [jt] * ((n - 512) // jt)
        taper = [jt] * ((n - 1024) // jt) + [512, 256, 256]
    elif kind == "ramp256":
        ramp = [256, 256] + [jt] * ((n - 512) // jt)
        taper = [jt] * ((n - 512) // jt) + [256, 256]
    elif kind == "tail128":
        ramp = [256, 256] + [jt] * ((n - 512) // jt)
        taper = [jt] * ((n - 512) // jt) + [256, 128, 128]
    else:
        raise ValueError(kind)
    assert sum(ramp) == n and sum(taper) == n
    return ramp, base, taper


def build(n=4096, de=16, df=128, n_cores=N_CORES, jt=1024, reps=1,
          ebufs=None, sbufs=4, sched="ramp256", gp_k=0, dummy="none",
          mode="psplit", copy_eng="scalar", kd=8):
    """Build the per-core Bass program. Returns nc.

    gp_k: number of trailing k-steps of the score chain offloaded to the
    GPSIMD (Pool) engine into a separate partial, combined by one DVE add.

    dummy: benchmarking diagnostic only (kernel() always uses "none").
    For reps > 1, all but the LAST rep are built with identical instruction
    structure/count but altered access-pattern sizes, so per-call host
    overhead (which scales with program size) cancels in a reps-slope while
    device-exec content changes:
      "compute": DVE/ACT ops shrunk to 16-wide, edge DMAs kept FULL size
                 -> extra reps cost ~ the pure DMA stream.
      "tiny":    edge DMAs shrunk to 16-wide too -> extra reps cost ~
                 instruction-issue only.
    The final rep is always the real computation, so outputs stay correct.
    """
    if ebufs is None:
        # as many edge-tile buffers as fit beside ~30KB/partition of other
        # tiles (jt=1024 -> 2, jt=512 -> 5)
        ebufs = max(2, (208 - 30) * 1024 // (jt * de * 4))
    rows = n // n_cores          # i-rows per core
    iblk = 128                   # i-rows per block (partition dim)
    nblk = rows // iblk
    nck = n // 128               # 128-wide j chunks (for transpose/aggregation)
    if mode == "ttr":
        ebufs = min(ebufs, 4)    # make room for the ttr scratch tile

    nc = bacc.Bacc("TRN2", target_bir_lowering=False, debug=False)
    ed = nc.dram_tensor("edges", [rows, n, de], F32, kind="ExternalInput")
    ft = nc.dram_tensor("features", [n, df], F32, kind="ExternalInput")
    wd = nc.dram_tensor("W", [de, 1], F32, kind="ExternalInput")
    out = nc.dram_tensor("out", [rows, df], F32, kind="ExternalOutput")

    with tile.TileContext(nc) as tc, ExitStack() as ctx:
        consts = ctx.enter_context(tc.tile_pool(name="consts", bufs=1))
        epool = ctx.enter_context(tc.tile_pool(name="epool", bufs=ebufs))
        spool = ctx.enter_context(tc.tile_pool(name="spool", bufs=sbufs))
        if gp_k:
            gpool = ctx.enter_context(tc.tile_pool(name="gpool", bufs=sbufs))
        if mode == "ttr":
            xpool = ctx.enter_context(tc.tile_pool(name="xpool", bufs=1))
        if mode == "ilv2":
            i2pool = ctx.enter_context(tc.tile_pool(name="i2pool", bufs=2))
        if mode == "psplit":
            # ACT-product scratch (SBUF) + PE k-accumulator (PSUM, 2 banks)
            prodpool = ctx.enter_context(tc.tile_pool(name="prodpool", bufs=2))
            qpool = ctx.enter_context(
                tc.tile_pool(name="qpool", bufs=2, space="PSUM"))
        tpool = ctx.enter_context(tc.tile_pool(name="tpool", bufs=4))
        mpool = ctx.enter_context(tc.tile_pool(name="mpool", bufs=4))
        opool = ctx.enter_context(tc.tile_pool(name="opool", bufs=2))
        ppool = ctx.enter_context(tc.tile_pool(
            name="ppool", bufs=2 if mode == "psplit" else 4, space="PSUM"))
        upool = ctx.enter_context(tc.tile_pool(name="upool", bufs=2, space="PSUM"))

        # --- constants -----------------------------------------------------
        ramp, base, taper = _schedule(n, jt, sched)
        dve_k = de - gp_k

        ident = consts.tile([128, 128], F32)
        make_identity(nc, ident)

        # features chunks: featx[p, c, d] = features[128c + p, d]
        # (ACT-ring DMA: the 2MB load never delays the edge stream on SP)
        featx = consts.tile([128, nck, df], F32)
        nc.scalar.dma_start(
            out=featx[:], in_=ft.rearrange("(c p) d -> p c d", p=128)
        )

        # W broadcast to all partitions: wall[p, k] = W[k]
        # (copied through DVE so the hot TT mul only waits on the edges DMA —
        # the 3-operand TT ISA encoding has a single sync-wait slot)
        wall_raw = consts.tile([128, de], F32)
        nc.sync.dma_start(out=wall_raw[:], in_=bass.AP(wd, 0, [[0, 128], [1, de]]))
        wall = consts.tile([128, de], F32)
        nc.vector.tensor_copy(wall[:], wall_raw[:])
        if gp_k:
            wall_gp = consts.tile([128, de], F32)
            nc.vector.tensor_copy(wall_gp[:], wall_raw[:])
        if mode == "psplit":
            wall_act = consts.tile([128, de], F32)
            nc.vector.tensor_copy(wall_act[:], wall_raw[:])
        if mode == "ttr":
            # scratch for tensor_tensor_reduce's product output. bufs=1 is
            # safe: nothing reads it, and the WAW dependency just serializes
            # the ttr ops, which run serially on DVE anyway.
            xscr = xpool.tile([128, jt, de], F32)

        # --- main loop -----------------------------------------------------
        for _rep in range(reps):
          is_dummy = dummy != "none" and _rep != reps - 1
          for b in range(nblk):
              sched_b = ramp if b == 0 else (taper if b == nblk - 1 else base)
              nzp = len(sched_b)
              zparts = mpool.tile([128, nzp], F32, tag="zparts")
              pu = upool.tile([128, df], F32)
              j0 = 0
              for q, sz in enumerate(sched_b):
                  csz = sz if dummy == "dmatiny" else (16 if is_dummy else sz)
                  dsz = (16 if dummy in ("tiny", "dmatiny") else sz) \
                      if is_dummy else sz
                  et = epool.tile([128, jt, de], F32)
                  nc.sync.dma_start(
                      out=et[:, 0:dsz, :],
                      in_=ed[b * iblk:(b + 1) * iblk, j0:j0 + dsz, :],
                  )
                  # scores_slice = sum_k E[:, :, k] * W[k]
                  stile = spool.tile([128, jt], F32, tag="stile")
                  ssl = stile[:, 0:csz]
                  if mode == "ttr":
                      # ONE fused DVE op per tile: product with broadcast W
                      # (contiguous reads) + grouped row-sum over k into the
                      # scores slice. 16x fewer DVE instructions than the
                      # scalar_tensor_tensor chain and no strided reads.
                      wbc = wall.rearrange("p k -> p () k").broadcast_to(
                          [128, csz, de])
                      nc.vector.tensor_tensor_reduce(
                          out=xscr[:, 0:csz, :], in0=et[:, 0:csz, :], in1=wbc,
                          scale=1.0, scalar=0.0, op0=ALU.mult, op1=ALU.add,
                          accum_out=stile.rearrange("p j -> p j ()")[:, 0:csz, :],
                      )
                  elif mode == "psplit":
                      # k-contraction split across three engines so no single
                      # engine's DRAIN-doubled throughput gates the DMA
                      # stream:  DVE chains k<kd; ACT forms w_k*E_k products
                      # for k>=kd which PE accumulates (identity-stationary
                      # matmuls) into a 2-bank PSUM tile; PE also folds the
                      # DVE partial in, and tanh reads straight from PSUM.
                      nc.vector.tensor_scalar(
                          ssl, et[:, 0:csz, 0], wall[:, 0:1], None, ALU.mult)
                      for k in range(1, kd):
                          nc.vector.scalar_tensor_tensor(
                              ssl, et[:, 0:csz, k], wall[:, k:k + 1], ssl,
                              ALU.mult, ALU.add,
                          )
                      ps = qpool.tile([128, jt], F32, tag="ps")
                      mid = csz // 2 if csz <= 512 else 512
                      pieces = [(0, mid), (mid, csz)]
                      for ki, k in enumerate(range(kd, de)):
                          pk = prodpool.tile([128, jt], F32, tag="pk")
                          nc.scalar.activation(
                              pk[:, 0:csz], et[:, 0:csz, k], AF.Copy,
                              scale=wall_act[:, k:k + 1])
                          for pi, (a, e) in enumerate(pieces):
                              # start clears the has-written bits bank-wide;
                              # only the first touch of each bank sets it
                              nc.tensor.matmul(
                                  ps[:, a:e], ident[:], pk[:, a:e],
                                  start=(ki == 0 and (pi == 0 or a == 512)),
                                  stop=False,
                              )
                      for pi, (a, e) in enumerate(pieces):
                          nc.tensor.matmul(
                              ps[:, a:e], ident[:], stile[:, a:e],
                              start=False, stop=True,
                          )
                  elif mode == "ilv2":
                      # two independent interleaved partial chains: even k
                      # accumulate into ssl, odd k into p2sl.  Consecutive
                      # DVE instructions then never have a RAW dependency,
                      # so the engine pipeline never drains mid-chain.
                      # One extra combine add at the end.
                      p2 = i2pool.tile([128, jt], F32, tag="p2")
                      p2sl = p2[:, 0:csz]
                      nc.vector.tensor_scalar(
                          ssl, et[:, 0:csz, 0], wall[:, 0:1], None, ALU.mult)
                      nc.vector.tensor_scalar(
                          p2sl, et[:, 0:csz, 1], wall[:, 1:2], None, ALU.mult)
                      for k in range(2, dve_k):
                          dst = ssl if k % 2 == 0 else p2sl
                          nc.vector.scalar_tensor_tensor(
                              dst, et[:, 0:csz, k], wall[:, k:k + 1], dst,
                              ALU.mult, ALU.add,
                          )
                      nc.vector.tensor_tensor(ssl, p2sl, ssl, ALU.add)
                  else:
                      # chain of 16 in-place scalar_tensor_tensor ops
                      # (scalar = per-partition W[k] broadcast)
                      nc.vector.tensor_scalar(
                          ssl, et[:, 0:csz, 0], wall[:, 0:1], None, ALU.mult)
                      for k in range(1, dve_k):
                          nc.vector.scalar_tensor_tensor(
                              ssl, et[:, 0:csz, k], wall[:, k:k + 1], ssl,
                              ALU.mult, ALU.add,
                          )
                  if gp_k:
                      gtile = gpool.tile([128, jt], F32, tag="gtile")
                      gsl = gtile[:, 0:csz]
                      nc.gpsimd.tensor_scalar(
                          gsl, et[:, 0:csz, dve_k], wall_gp[:, dve_k:dve_k + 1],
                          None, ALU.mult)
                      for k in range(dve_k + 1, de):
                          nc.gpsimd.scalar_tensor_tensor(
                              gsl, et[:, 0:csz, k], wall_gp[:, k:k + 1], gsl,
                              ALU.mult, ALU.add,
                          )
                      nc.vector.tensor_tensor(ssl, gsl, ssl, ALU.add)
                  # per-slice softmax numerator + partial row-sum
                  if mode == "psplit":
                      nc.scalar.activation(ssl, ps[:, 0:csz], AF.Tanh)
                  else:
                      nc.scalar.activation(ssl, ssl, AF.Tanh)
                  nc.scalar.activation(ssl, ssl, AF.Exp,
                                       accum_out=zparts[:, q:q + 1])
                  # transpose + aggregate this slice's 128-chunks immediately
                  for ci in range(sz // 128):
                      c = j0 // 128 + ci
                      ptile = ppool.tile([128, 128], F32)
                      nc.tensor.transpose(
                          ptile[:], stile[:, ci * 128:(ci + 1) * 128], ident[:]
                      )
                      atc = tpool.tile([128, 128], F32)
                      if copy_eng == "gpsimd":
                          # PSUM->SBUF staging on the otherwise-idle Pool
                          # engine, keeping ACT free for tanh/exp
                          nc.gpsimd.tensor_copy(atc[:], ptile[:])
                      elif copy_eng == "mixed":
                          # alternate DVE/ACT so neither eats all the copies
                          if ci % 2 == 0:
                              nc.vector.tensor_copy(atc[:], ptile[:])
                          else:
                              nc.scalar.copy(atc[:], ptile[:])
                      elif copy_eng == "vector":
                          nc.vector.tensor_copy(atc[:], ptile[:])
                      else:
                          nc.scalar.copy(atc[:], ptile[:])
                      nc.tensor.matmul(
                          pu[:], atc[:], featx[:, c, :],
                          start=(c == 0), stop=(c == nck - 1),
                      )
                  j0 += sz

              zcol = mpool.tile([128, 1], F32)
              nc.vector.reduce_sum(out=zcol[:], in_=zparts[:], axis=AX.X)
              rz = mpool.tile([128, 1], F32)
              nc.vector.reciprocal(rz[:], zcol[:])

              # out = tanh(pu * rz)
              osb = opool.tile([128, df], F32)
              nc.scalar.activation(osb[:], pu[:], AF.Tanh, scale=rz[:])
              # out-DMA on the scalar-engine HWDGE ring: edge DMAs (SP ring)
              # never queue behind it (HWDGE is FIFO per issuing engine)
              nc.scalar.dma_start(
                  out=out[b * iblk:(b + 1) * iblk, :], in_=osb[:]
              )

    nc.compile()
    return nc


_CACHE = {}


def _get_nc(shape_key):
    if shape_key not in _CACHE:
        n, de, df = shape_key
        _CACHE[shape_key] = build(n=n, de=de, df=df)
    return _CACHE[shape_key]


def kernel(edges, features, W):
    n, n2, de = edges.shape
    df = features.shape[1]
    rows = n // N_CORES
    nc = _get_nc((n, de, df))

    edges = np.ascontiguousarray(edges, dtype=np.float32)
    features = np.ascontiguousarray(features, dtype=np.float32)
    W = np.ascontiguousarray(W, dtype=np.float32)

    in_maps = [
        {
            "edges": edges[c * rows:(c + 1) * rows],
            "features": features,
            "W": W,
        }
        for c in range(N_CORES)
    ]
    res = run_bass_kernel_spmd(nc, in_maps, core_ids=list(range(N_CORES)))
    return np.concatenate([r["out"] for r in res.results], axis=0)



# revision 20
# speedup vs baseline: 1.5986x; 1.2235x over previous
"""GAT-style attention kernel for Trainium2 (8 NeuronCores, row-parallel).

Computation (per reference):
    scores    = tanh(einsum('ijk,ko->ijo', edges, W))        # (N, N, 1)
    attention = softmax(scores, axis=1).reshape(N, N)        # row softmax over j
    out       = tanh(attention @ features)                   # (N, D_FEAT)

Sharding: rows (i axis) split across the 8 cores; features/W replicated; no
cross-core communication (each row's softmax + aggregation is local).

The kernel is HBM-bound: the 1.07GB edges tensor streams once, 134.2MB per
core.  HW-measured stream rate for this access pattern (one SP-ring HWDGE
DMA per [128 x 1024 x 16] tile, 64KB contiguous per partition) is
~368 GB/s/core == the HBM-per-NC wall (two-ring alternation measured
SLOWER, 337 GB/s), so the whole job is hiding ALL compute behind a ~23us
per-tile DMA window.

The k-contraction (scores = sum_k E_k*W[k]) is the expensive part: every
DVE op pays an unavoidable post-op DRAIN (~= op_duration - 266ns, an
output-hazard pipe flush), so a 16-op in-place chain runs at HALF the DVE's
element rate (~22.9us/tile, HW-measured) and ties the DMA window.  The fix
is splitting the contraction across three engines (mode="psplit", kd=10):
  1. DMA   edges tile (128 i, 1024 j, 16 k) on the sync-engine ring;
           featx/W preloads go on the ACT ring so the edge stream starts
           at t=0.
  2. DVE   partial scores for k < kd: chain of kd scalar_tensor_tensor ops
           (~14us/tile with DRAIN).
  3. ACT   products p_k = W[k]*E_k for k >= kd via activation(Copy,
           scale=W[k] per-partition broadcast), ~1.1us each.
  4. PE    accumulates the p_k into a 2-bank PSUM tile via
           identity-stationary matmuls (2 x 512-col per k), then folds the
           DVE partial in the same way (start clears the has-written bits
           of each bank once; everything else accumulates).
  5. ACT   tanh reads the summed scores STRAIGHT FROM PSUM -> stile (SBUF),
           then exp in place; exp's accum_out collects row-sum partials of Z.
  6. PE    transposes each 128x128 att chunk into a shared PSUM bank
           (pack_t=4 chunks per bank = 2KB exactly); ONE wide ACT copy
           stages all 4 back to SBUF (8 copies/tile -> 2); PE accumulates
           attT_chunk.T @ features_chunk into the PSUM output tile.
  7. DVE   Z = sum of partials; rz = 1/Z;  ACT out = tanh(psum * rz);
           out-DMA on the ACT HWDGE ring (never queues the edge stream).

Per-tile engine busy (HW-calibrated): DMA 23.4us, DVE ~14us, ACT ~10us,
PE ~5us -- DMA-bound with margin on every compute engine.

Numerics: softmax skips the max-subtraction (scores are tanh-bounded in
(-1,1) so exp cannot overflow) and normalization is folded to the end
(aggregation is linear in att). All arithmetic fp32; HW rel err vs the jax
reference ~2.6e-6.

Rejected on HW measurement: tensor_tensor_reduce (accum_out must be
free_size=1 -- cannot do grouped per-j reduce), PE products from strided
et[:, :, k] rhs with scaled-identity stationaries (pe_prod=True, +70us:
strided rhs streams slowly), DVE/gpsimd PSUM->SBUF copies (DVE read vs PE
write hangs the device; Pool cannot reach PSUM), two-ring edge DMA (HBM
wall, not ring-bound), taper schedules (more small ops = net loss).
"""

from contextlib import ExitStack

import numpy as np

import concourse.bass as bass
import concourse.bacc as bacc
import concourse.tile as tile
from concourse import mybir
from concourse.bass_utils import run_bass_kernel_spmd
from concourse.masks import make_identity

F32 = mybir.dt.float32
AF = mybir.ActivationFunctionType
ALU = mybir.AluOpType
AX = mybir.AxisListType

N_CORES = 8


def _schedule(n, jt, kind):
    """Per-block j-tile size lists (ramp for block 0, taper for the last)."""
    base = [jt] * (n // jt)
    if kind == "tail1024" and jt == 1024 and n >= 4096:
        # taper the final block so the post-DMA compute drain is short
        taper = [jt] * ((n - 1024) // jt) + [512, 256, 128, 128]
        assert sum(taper) == n
        return base, base, taper
    if kind == "uniform" or jt != 512 or n < 2048:
        return base, base, base
    if kind == "ramp":
        ramp = [128, 128, 256] + [jt] * ((n - 512) // jt)
        taper = [jt] * ((n - 1024) // jt) + [512, 256, 256]
    elif kind == "ramp256":
        ramp = [256, 256] + [jt] * ((n - 512) // jt)
        taper = [jt] * ((n - 512) // jt) + [256, 256]
    elif kind == "tail128":
        ramp = [256, 256] + [jt] * ((n - 512) // jt)
        taper = [jt] * ((n - 512) // jt) + [256, 128, 128]
    else:
        raise ValueError(kind)
    assert sum(ramp) == n and sum(taper) == n
    return ramp, base, taper


def build(n=4096, de=16, df=128, n_cores=N_CORES, jt=1024, reps=1,
          ebufs=None, sbufs=4, sched="ramp256", gp_k=0, dummy="none",
          mode="psplit", copy_eng="scalar", kd=10, pack_t=4, pe_prod=False):
    """Build the per-core Bass program. Returns nc.

    gp_k: number of trailing k-steps of the score chain offloaded to the
    GPSIMD (Pool) engine into a separate partial, combined by one DVE add.

    dummy: benchmarking diagnostic only (kernel() always uses "none").
    For reps > 1, all but the LAST rep are built with identical instruction
    structure/count but altered access-pattern sizes, so per-call host
    overhead (which scales with program size) cancels in a reps-slope while
    device-exec content changes:
      "compute": DVE/ACT ops shrunk to 16-wide, edge DMAs kept FULL size
                 -> extra reps cost ~ the pure DMA stream.
      "tiny":    edge DMAs shrunk to 16-wide too -> extra reps cost ~
                 instruction-issue only.
    The final rep is always the real computation, so outputs stay correct.
    """
    if ebufs is None:
        # as many edge-tile buffers as fit beside ~30KB/partition of other
        # tiles (jt=1024 -> 2, jt=512 -> 5)
        ebufs = max(2, (208 - 30) * 1024 // (jt * de * 4))
    rows = n // n_cores          # i-rows per core
    iblk = 128                   # i-rows per block (partition dim)
    nblk = rows // iblk
    nck = n // 128               # 128-wide j chunks (for transpose/aggregation)
    if mode == "ttr":
        ebufs = min(ebufs, 4)    # make room for the ttr scratch tile

    nc = bacc.Bacc("TRN2", target_bir_lowering=False, debug=False)
    ed = nc.dram_tensor("edges", [rows, n, de], F32, kind="ExternalInput")
    ft = nc.dram_tensor("features", [n, df], F32, kind="ExternalInput")
    wd = nc.dram_tensor("W", [de, 1], F32, kind="ExternalInput")
    out = nc.dram_tensor("out", [rows, df], F32, kind="ExternalOutput")

    with tile.TileContext(nc) as tc, ExitStack() as ctx:
        consts = ctx.enter_context(tc.tile_pool(name="consts", bufs=1))
        epool = ctx.enter_context(tc.tile_pool(name="epool", bufs=ebufs))
        spool = ctx.enter_context(tc.tile_pool(name="spool", bufs=sbufs))
        if gp_k:
            gpool = ctx.enter_context(tc.tile_pool(name="gpool", bufs=sbufs))
        if mode == "ttr":
            xpool = ctx.enter_context(tc.tile_pool(name="xpool", bufs=1))
        if mode == "ilv2":
            i2pool = ctx.enter_context(tc.tile_pool(name="i2pool", bufs=2))
        if mode == "psplit":
            # ACT-product scratch (SBUF) + PE k-accumulator (PSUM, 2 banks)
            prodpool = ctx.enter_context(tc.tile_pool(name="prodpool", bufs=2))
            qpool = ctx.enter_context(
                tc.tile_pool(name="qpool", bufs=2, space="PSUM"))
        tpool = ctx.enter_context(tc.tile_pool(name="tpool", bufs=4))
        mpool = ctx.enter_context(tc.tile_pool(name="mpool", bufs=4))
        opool = ctx.enter_context(tc.tile_pool(name="opool", bufs=2))
        ppool = ctx.enter_context(tc.tile_pool(
            name="ppool", bufs=2 if mode == "psplit" else 4, space="PSUM"))
        upool = ctx.enter_context(tc.tile_pool(name="upool", bufs=2, space="PSUM"))

        # --- constants -----------------------------------------------------
        ramp, base, taper = _schedule(n, jt, sched)
        dve_k = de - gp_k

        ident = consts.tile([128, 128], F32)
        make_identity(nc, ident)

        # features chunks: featx[p, c, d] = features[128c + p, d]
        # (ACT-ring DMA: the 2MB load never delays the edge stream on SP)
        featx = consts.tile([128, nck, df], F32)
        nc.scalar.dma_start(
            out=featx[:], in_=ft.rearrange("(c p) d -> p c d", p=128)
        )

        # W broadcast to all partitions: wall[p, k] = W[k]
        # (copied through DVE so the hot TT mul only waits on the edges DMA —
        # the 3-operand TT ISA encoding has a single sync-wait slot)
        wall_raw = consts.tile([128, de], F32)
        nc.sync.dma_start(out=wall_raw[:], in_=bass.AP(wd, 0, [[0, 128], [1, de]]))
        wall = consts.tile([128, de], F32)
        nc.vector.tensor_copy(wall[:], wall_raw[:])
        if gp_k:
            wall_gp = consts.tile([128, de], F32)
            nc.vector.tensor_copy(wall_gp[:], wall_raw[:])
        if mode == "psplit":
            wall_act = consts.tile([128, de], F32)
            nc.vector.tensor_copy(wall_act[:], wall_raw[:])
            if pe_prod:
                # scaled identities w_k*I: PE stationaries that scale AND
                # accumulate E[:, :, k] into PSUM in one matmul each
                wident = consts.tile([128, de, 128], F32)
                for k in range(kd, de):
                    nc.vector.tensor_scalar(
                        wident[:, k, :], ident[:], wall[:, k:k + 1], None,
                        ALU.mult)
        if mode == "ttr":
            # scratch for tensor_tensor_reduce's product output. bufs=1 is
            # safe: nothing reads it, and the WAW dependency just serializes
            # the ttr ops, which run serially on DVE anyway.
            xscr = xpool.tile([128, jt, de], F32)

        # --- main loop -----------------------------------------------------
        for _rep in range(reps):
          is_dummy = dummy != "none" and _rep != reps - 1
          for b in range(nblk):
              sched_b = ramp if b == 0 else (taper if b == nblk - 1 else base)
              nzp = len(sched_b)
              zparts = mpool.tile([128, nzp], F32, tag="zparts")
              pu = upool.tile([128, df], F32)
              j0 = 0
              if is_dummy and dummy.startswith("dmaonly"):
                  # pure-DMA reps: only the edge loads, no compute at all --
                  # measures the HBM stream rate for this access pattern.
                  # dmaonly2*: alternate SP/ACT HWDGE rings per tile.
                  # dmaonly2h*: both rings, half-tiles each (2 DMAs/tile).
                  j0 = 0
                  for q, sz in enumerate(sched_b):
                      dsz = 16 if dummy.endswith("_tiny") else sz
                      et = epool.tile([128, jt, de], F32)
                      if "2h" in dummy:
                          h = dsz // 2
                          nc.sync.dma_start(
                              out=et[:, 0:h, :],
                              in_=ed[b * iblk:(b + 1) * iblk, j0:j0 + h, :],
                          )
                          nc.scalar.dma_start(
                              out=et[:, h:dsz, :],
                              in_=ed[b * iblk:(b + 1) * iblk,
                                     j0 + h:j0 + dsz, :],
                          )
                      else:
                          eng = (nc.scalar if ("2" in dummy and q % 2)
                                 else nc.sync)
                          eng.dma_start(
                              out=et[:, 0:dsz, :],
                              in_=ed[b * iblk:(b + 1) * iblk, j0:j0 + dsz, :],
                          )
                      j0 += sz
                  continue
              for q, sz in enumerate(sched_b):
                  csz = sz if dummy == "dmatiny" else (16 if is_dummy else sz)
                  dsz = (16 if dummy in ("tiny", "dmatiny") else sz) \
                      if is_dummy else sz
                  et = epool.tile([128, jt, de], F32)
                  nc.sync.dma_start(
                      out=et[:, 0:dsz, :],
                      in_=ed[b * iblk:(b + 1) * iblk, j0:j0 + dsz, :],
                  )
                  # scores_slice = sum_k E[:, :, k] * W[k]
                  stile = spool.tile([128, jt], F32, tag="stile")
                  ssl = stile[:, 0:csz]
                  if mode == "ttr":
                      # ONE fused DVE op per tile: product with broadcast W
                      # (contiguous reads) + grouped row-sum over k into the
                      # scores slice. 16x fewer DVE instructions than the
                      # scalar_tensor_tensor chain and no strided reads.
                      wbc = wall.rearrange("p k -> p () k").broadcast_to(
                          [128, csz, de])
                      nc.vector.tensor_tensor_reduce(
                          out=xscr[:, 0:csz, :], in0=et[:, 0:csz, :], in1=wbc,
                          scale=1.0, scalar=0.0, op0=ALU.mult, op1=ALU.add,
                          accum_out=stile.rearrange("p j -> p j ()")[:, 0:csz, :],
                      )
                  elif mode == "psplit":
                      # k-contraction split across three engines so no single
                      # engine's DRAIN-doubled throughput gates the DMA
                      # stream:  DVE chains k<kd; ACT forms w_k*E_k products
                      # for k>=kd which PE accumulates (identity-stationary
                      # matmuls) into a 2-bank PSUM tile; PE also folds the
                      # DVE partial in, and tanh reads straight from PSUM.
                      if kd > 0:
                          nc.vector.tensor_scalar(
                              ssl, et[:, 0:csz, 0], wall[:, 0:1], None,
                              ALU.mult)
                      for k in range(1, kd):
                          nc.vector.scalar_tensor_tensor(
                              ssl, et[:, 0:csz, k], wall[:, k:k + 1], ssl,
                              ALU.mult, ALU.add,
                          )
                      ps = qpool.tile([128, jt], F32, tag="ps")
                      mid = csz // 2 if csz <= 512 else 512
                      pieces = [(0, mid), (mid, csz)]
                      for ki, k in enumerate(range(kd, de)):
                          if pe_prod:
                              # lhsT = w_k*I -> ps += w_k * E[:, :, k]
                              for pi, (a, e) in enumerate(pieces):
                                  nc.tensor.matmul(
                                      ps[:, a:e], wident[:, k, :],
                                      et[:, a:e, k],
                                      start=(ki == 0 and (pi == 0 or a == 512)),
                                      stop=(kd == 0 and k == de - 1),
                                  )
                              continue
                          pk = prodpool.tile([128, jt], F32, tag="pk")
                          nc.scalar.activation(
                              pk[:, 0:csz], et[:, 0:csz, k], AF.Copy,
                              scale=wall_act[:, k:k + 1])
                          for pi, (a, e) in enumerate(pieces):
                              # start clears the has-written bits bank-wide;
                              # only the first touch of each bank sets it
                              nc.tensor.matmul(
                                  ps[:, a:e], ident[:], pk[:, a:e],
                                  start=(ki == 0 and (pi == 0 or a == 512)),
                                  stop=False,
                              )
                      for pi, (a, e) in enumerate(pieces):
                          if kd > 0:
                              nc.tensor.matmul(
                                  ps[:, a:e], ident[:], stile[:, a:e],
                                  start=False, stop=True,
                              )
                  elif mode == "ilv2":
                      # two independent interleaved partial chains: even k
                      # accumulate into ssl, odd k into p2sl.  Consecutive
                      # DVE instructions then never have a RAW dependency,
                      # so the engine pipeline never drains mid-chain.
                      # One extra combine add at the end.
                      p2 = i2pool.tile([128, jt], F32, tag="p2")
                      p2sl = p2[:, 0:csz]
                      nc.vector.tensor_scalar(
                          ssl, et[:, 0:csz, 0], wall[:, 0:1], None, ALU.mult)
                      nc.vector.tensor_scalar(
                          p2sl, et[:, 0:csz, 1], wall[:, 1:2], None, ALU.mult)
                      for k in range(2, dve_k):
                          dst = ssl if k % 2 == 0 else p2sl
                          nc.vector.scalar_tensor_tensor(
                              dst, et[:, 0:csz, k], wall[:, k:k + 1], dst,
                              ALU.mult, ALU.add,
                          )
                      nc.vector.tensor_tensor(ssl, p2sl, ssl, ALU.add)
                  else:
                      # chain of 16 in-place scalar_tensor_tensor ops
                      # (scalar = per-partition W[k] broadcast)
                      nc.vector.tensor_scalar(
                          ssl, et[:, 0:csz, 0], wall[:, 0:1], None, ALU.mult)
                      for k in range(1, dve_k):
                          nc.vector.scalar_tensor_tensor(
                              ssl, et[:, 0:csz, k], wall[:, k:k + 1], ssl,
                              ALU.mult, ALU.add,
                          )
                  if gp_k:
                      gtile = gpool.tile([128, jt], F32, tag="gtile")
                      gsl = gtile[:, 0:csz]
                      nc.gpsimd.tensor_scalar(
                          gsl, et[:, 0:csz, dve_k], wall_gp[:, dve_k:dve_k + 1],
                          None, ALU.mult)
                      for k in range(dve_k + 1, de):
                          nc.gpsimd.scalar_tensor_tensor(
                              gsl, et[:, 0:csz, k], wall_gp[:, k:k + 1], gsl,
                              ALU.mult, ALU.add,
                          )
                      nc.vector.tensor_tensor(ssl, gsl, ssl, ALU.add)
                  # per-slice softmax numerator + partial row-sum
                  if mode == "psplit":
                      nc.scalar.activation(ssl, ps[:, 0:csz], AF.Tanh)
                  else:
                      nc.scalar.activation(ssl, ssl, AF.Tanh)
                  nc.scalar.activation(ssl, ssl, AF.Exp,
                                       accum_out=zparts[:, q:q + 1])
                  # transpose + aggregate this slice's 128-chunks; pack_t
                  # chunks share one PSUM bank so ONE wide ACT copy stages
                  # them all (fewer per-op overheads on the ACT engine)
                  nchunk = sz // 128
                  ci = 0
                  while ci < nchunk:
                      g = min(pack_t, nchunk - ci, 4)
                      ptile = ppool.tile([128, 128 * pack_t], F32, tag="pt")
                      for u in range(g):
                          nc.tensor.transpose(
                              ptile[:, u * 128:(u + 1) * 128],
                              stile[:, (ci + u) * 128:(ci + u + 1) * 128],
                              ident[:],
                          )
                      atc = tpool.tile([128, 128 * pack_t], F32, tag="atc")
                      if copy_eng == "gpsimd":
                          # PSUM->SBUF staging on the otherwise-idle Pool
                          # engine, keeping ACT free for tanh/exp
                          nc.gpsimd.tensor_copy(
                              atc[:, 0:128 * g], ptile[:, 0:128 * g])
                      else:
                          nc.scalar.copy(
                              atc[:, 0:128 * g], ptile[:, 0:128 * g])
                      for u in range(g):
                          c = j0 // 128 + ci + u
                          nc.tensor.matmul(
                              pu[:], atc[:, u * 128:(u + 1) * 128],
                              featx[:, c, :],
                              start=(c == 0), stop=(c == nck - 1),
                          )
                      ci += g
                  j0 += sz

              zcol = mpool.tile([128, 1], F32)
              nc.vector.reduce_sum(out=zcol[:], in_=zparts[:], axis=AX.X)
              rz = mpool.tile([128, 1], F32)
              nc.vector.reciprocal(rz[:], zcol[:])

              # out = tanh(pu * rz)
              osb = opool.tile([128, df], F32)
              nc.scalar.activation(osb[:], pu[:], AF.Tanh, scale=rz[:])
              # out-DMA on the scalar-engine HWDGE ring: edge DMAs (SP ring)
              # never queue behind it (HWDGE is FIFO per issuing engine)
              nc.scalar.dma_start(
                  out=out[b * iblk:(b + 1) * iblk, :], in_=osb[:]
              )

    nc.compile()
    return nc


_CACHE = {}


def _get_nc(shape_key):
    if shape_key not in _CACHE:
        n, de, df = shape_key
        _CACHE[shape_key] = build(n=n, de=de, df=df)
    return _CACHE[shape_key]


def kernel(edges, features, W):
    n, n2, de = edges.shape
    df = features.shape[1]
    rows = n // N_CORES
    nc = _get_nc((n, de, df))

    edges = np.ascontiguousarray(edges, dtype=np.float32)
    features = np.ascontiguousarray(features, dtype=np.float32)
    W = np.ascontiguousarray(W, dtype=np.float32)

    in_maps = [
        {
            "edges": edges[c * rows:(c + 1) * rows],
            "features": features,
            "W": W,
        }
        for c in range(N_CORES)
    ]
    res = run_bass_kernel_spmd(nc, in_maps, core_ids=list(range(N_CORES)))
    return np.concatenate([r["out"] for r in res.results], axis=0)



# revision 31
# speedup vs baseline: 2.2412x; 1.4019x over previous
"""GAT-style attention kernel for Trainium2 (8 NeuronCores, row-parallel).

Computation (per reference):
    scores    = tanh(einsum('ijk,ko->ijo', edges, W))        # (N, N, 1)
    attention = softmax(scores, axis=1).reshape(N, N)        # row softmax over j
    out       = tanh(attention @ features)                   # (N, D_FEAT)

Sharding: rows (i axis) split across the 8 cores; features/W replicated; no
cross-core communication (each row's softmax + aggregation is local).

The kernel is HBM-bound: the 1.07GB edges tensor streams once, 134.2MB per
core.  HW-measured stream rate for this access pattern (one SP-ring HWDGE
DMA per [128 x 1024 x 16] tile, 64KB contiguous per partition) is
~368 GB/s/core == the HBM-per-NC wall (two-ring alternation measured
SLOWER, 337 GB/s), so the whole job is hiding ALL compute behind a ~23us
per-tile DMA window.

The k-contraction (scores = sum_k E_k*W[k]) is the expensive part: every
DVE op pays an unavoidable post-op DRAIN (~= op_duration - 266ns, an
output-hazard pipe flush), so a 16-op in-place chain runs at HALF the DVE's
element rate (~22.9us/tile, HW-measured) and ties the DMA window.  The fix
is splitting the contraction across three engines (mode="psplit", kd=10):
  1. DMA   edges tile (128 i, 1024 j, 16 k) on the sync-engine ring;
           featx/W preloads go on the ACT ring so the edge stream starts
           at t=0.
  2. DVE   partial scores for k < kd: chain of kd scalar_tensor_tensor ops
           (~14us/tile with DRAIN).
  3. ACT   products p_k = W[k]*E_k for k >= kd via activation(Copy,
           scale=W[k] per-partition broadcast), ~1.1us each.
  4. PE    accumulates the p_k into a 2-bank PSUM tile via
           identity-stationary matmuls (2 x 512-col per k), then folds the
           DVE partial in the same way (start clears the has-written bits
           of each bank once; everything else accumulates).
  5. ACT   tanh reads the summed scores STRAIGHT FROM PSUM -> stile (SBUF),
           then exp in place; exp's accum_out collects row-sum partials of Z.
  6. PE    transposes each 128x128 att chunk into a shared PSUM bank
           (pack_t=4 chunks per bank = 2KB exactly); ONE wide ACT copy
           stages all 4 back to SBUF (8 copies/tile -> 2); PE accumulates
           attT_chunk.T @ features_chunk into the PSUM output tile.
  7. DVE   Z = sum of partials; rz = 1/Z;  ACT out = tanh(psum * rz);
           out-DMA on the ACT HWDGE ring (never queues the edge stream).

Per-tile engine busy (HW-calibrated): DMA 23.4us, DVE ~14us, ACT ~10us,
PE ~5us -- DMA-bound with margin on every compute engine.

Numerics: softmax skips the max-subtraction (scores are tanh-bounded in
(-1,1) so exp cannot overflow) and normalization is folded to the end
(aggregation is linear in att). All arithmetic fp32; HW rel err vs the jax
reference ~2.6e-6.

Rejected on HW measurement: tensor_tensor_reduce (accum_out must be
free_size=1 -- cannot do grouped per-j reduce), PE products from strided
et[:, :, k] rhs with scaled-identity stationaries (pe_prod=True, +70us:
strided rhs streams slowly), DVE/gpsimd PSUM->SBUF copies (DVE read vs PE
write hangs the device; Pool cannot reach PSUM), two-ring edge DMA (HBM
wall, not ring-bound), taper schedules (more small ops = net loss).
"""

from contextlib import ExitStack

import numpy as np

import concourse.bass as bass
import concourse.bacc as bacc
import concourse.tile as tile
from concourse import mybir
from concourse.bass_utils import run_bass_kernel_spmd
from concourse.masks import make_identity

F32 = mybir.dt.float32
BF16 = mybir.dt.bfloat16
AF = mybir.ActivationFunctionType
ALU = mybir.AluOpType
AX = mybir.AxisListType

N_CORES = 8
EDGE_DT = "bf16"         # stream edges at half the HBM bytes (arith fp32;
                         # HW rel err 9.2e-4 vs fp64, gate is 2e-2)


def _schedule(n, jt, kind):
    """Per-block j-tile size lists (ramp for block 0, taper for the last)."""
    base = [jt] * (n // jt)
    if kind == "tail1024" and jt == 1024 and n >= 4096:
        # taper the final block so the post-DMA compute drain is short
        taper = [jt] * ((n - 1024) // jt) + [512, 256, 128, 128]
        assert sum(taper) == n
        return base, base, taper
    if kind == "uniform" or jt != 512 or n < 2048:
        return base, base, base
    if kind == "ramp":
        ramp = [128, 128, 256] + [jt] * ((n - 512) // jt)
        taper = [jt] * ((n - 1024) // jt) + [512, 256, 256]
    elif kind == "ramp256":
        ramp = [256, 256] + [jt] * ((n - 512) // jt)
        taper = [jt] * ((n - 512) // jt) + [256, 256]
    elif kind == "tail128":
        ramp = [256, 256] + [jt] * ((n - 512) // jt)
        taper = [jt] * ((n - 512) // jt) + [256, 128, 128]
    else:
        raise ValueError(kind)
    assert sum(ramp) == n and sum(taper) == n
    return ramp, base, taper


def build(n=4096, de=16, df=128, n_cores=N_CORES, jt=1024, reps=1,
          ebufs=None, sbufs=4, sched="ramp256", gp_k=0, dummy="none",
          mode="psplit", copy_eng="scalar", kd=10, pack_t=4, pe_prod=False,
          edt="f32"):
    """Build the per-core Bass program. Returns nc.

    gp_k: number of trailing k-steps of the score chain offloaded to the
    GPSIMD (Pool) engine into a separate partial, combined by one DVE add.

    dummy: benchmarking diagnostic only (kernel() always uses "none").
    For reps > 1, all but the LAST rep are built with identical instruction
    structure/count but altered access-pattern sizes, so per-call host
    overhead (which scales with program size) cancels in a reps-slope while
    device-exec content changes:
      "compute": DVE/ACT ops shrunk to 16-wide, edge DMAs kept FULL size
                 -> extra reps cost ~ the pure DMA stream.
      "tiny":    edge DMAs shrunk to 16-wide too -> extra reps cost ~
                 instruction-issue only.
    The final rep is always the real computation, so outputs stay correct.
    """
    if ebufs is None:
        # as many edge-tile buffers as fit beside ~45KB/partition of other
        # tiles (f32 jt=1024 -> 2; bf16 jt=1024 -> 4)
        esz = jt * de * (2 if edt == "bf16" else 4)
        ebufs = min(4, max(2, (192 - 46) * 1024 // esz))
    rows = n // n_cores          # i-rows per core
    iblk = 128                   # i-rows per block (partition dim)
    nblk = rows // iblk
    nck = n // 128               # 128-wide j chunks (for transpose/aggregation)
    if mode == "ttr":
        ebufs = min(ebufs, 4)    # make room for the ttr scratch tile

    EDT = BF16 if edt == "bf16" else F32
    nc = bacc.Bacc("TRN2", target_bir_lowering=False, debug=False)
    ed = nc.dram_tensor("edges", [rows, n, de], EDT, kind="ExternalInput")
    ft = nc.dram_tensor("features", [n, df], F32, kind="ExternalInput")
    wd = nc.dram_tensor("W", [de, 1], F32, kind="ExternalInput")
    out = nc.dram_tensor("out", [rows, df], F32, kind="ExternalOutput")

    with tile.TileContext(nc) as tc, ExitStack() as ctx:
        consts = ctx.enter_context(tc.tile_pool(name="consts", bufs=1))
        epool = ctx.enter_context(tc.tile_pool(name="epool", bufs=ebufs))
        spool = ctx.enter_context(tc.tile_pool(name="spool", bufs=sbufs))
        if gp_k:
            gpool = ctx.enter_context(tc.tile_pool(name="gpool", bufs=sbufs))
        if mode == "ttr":
            xpool = ctx.enter_context(tc.tile_pool(name="xpool", bufs=1))
        if mode == "ilv2":
            i2pool = ctx.enter_context(tc.tile_pool(name="i2pool", bufs=2))
        if mode == "psplit":
            # ACT-product scratch (SBUF) + PE k-accumulator (PSUM, 2 banks)
            prodpool = ctx.enter_context(tc.tile_pool(name="prodpool", bufs=2))
            qpool = ctx.enter_context(
                tc.tile_pool(name="qpool", bufs=2, space="PSUM"))
        tpool = ctx.enter_context(tc.tile_pool(name="tpool", bufs=4))
        mpool = ctx.enter_context(tc.tile_pool(name="mpool", bufs=4))
        opool = ctx.enter_context(tc.tile_pool(name="opool", bufs=2))
        ppool = ctx.enter_context(tc.tile_pool(
            name="ppool", bufs=2 if mode == "psplit" else 4, space="PSUM"))
        upool = ctx.enter_context(tc.tile_pool(name="upool", bufs=2, space="PSUM"))

        # --- constants -----------------------------------------------------
        ramp, base, taper = _schedule(n, jt, sched)
        dve_k = de - gp_k

        ident = consts.tile([128, 128], F32)
        make_identity(nc, ident)

        # features chunks: featx[p, c, d] = features[128c + p, d]
        # (ACT-ring DMA: the 2MB load never delays the edge stream on SP)
        featx = consts.tile([128, nck, df], F32)
        nc.scalar.dma_start(
            out=featx[:], in_=ft.rearrange("(c p) d -> p c d", p=128)
        )

        # W broadcast to all partitions: wall[p, k] = W[k]
        # (copied through DVE so the hot TT mul only waits on the edges DMA —
        # the 3-operand TT ISA encoding has a single sync-wait slot)
        wall_raw = consts.tile([128, de], F32)
        nc.sync.dma_start(out=wall_raw[:], in_=bass.AP(wd, 0, [[0, 128], [1, de]]))
        wall = consts.tile([128, de], F32)
        nc.vector.tensor_copy(wall[:], wall_raw[:])
        if gp_k:
            wall_gp = consts.tile([128, de], F32)
            nc.vector.tensor_copy(wall_gp[:], wall_raw[:])
        if mode == "psplit":
            wall_act = consts.tile([128, de], F32)
            nc.vector.tensor_copy(wall_act[:], wall_raw[:])
            if pe_prod:
                # scaled identities w_k*I: PE stationaries that scale AND
                # accumulate E[:, :, k] into PSUM in one matmul each
                wident = consts.tile([128, de, 128], F32)
                for k in range(kd, de):
                    nc.vector.tensor_scalar(
                        wident[:, k, :], ident[:], wall[:, k:k + 1], None,
                        ALU.mult)
        if mode == "ttr":
            # scratch for tensor_tensor_reduce's product output. bufs=1 is
            # safe: nothing reads it, and the WAW dependency just serializes
            # the ttr ops, which run serially on DVE anyway.
            xscr = xpool.tile([128, jt, de], F32)

        # --- main loop -----------------------------------------------------
        for _rep in range(reps):
          is_dummy = dummy != "none" and _rep != reps - 1
          for b in range(nblk):
              sched_b = ramp if b == 0 else (taper if b == nblk - 1 else base)
              nzp = len(sched_b)
              zparts = mpool.tile([128, nzp], F32, tag="zparts")
              pu = upool.tile([128, df], F32)
              j0 = 0
              if is_dummy and dummy.startswith("dmaonly"):
                  # pure-DMA reps: only the edge loads, no compute at all --
                  # measures the HBM stream rate for this access pattern.
                  # dmaonly2*: alternate SP/ACT HWDGE rings per tile.
                  # dmaonly2h*: both rings, half-tiles each (2 DMAs/tile).
                  j0 = 0
                  for q, sz in enumerate(sched_b):
                      dsz = 16 if dummy.endswith("_tiny") else sz
                      et = epool.tile([128, jt, de], EDT)
                      if "2h" in dummy:
                          h = dsz // 2
                          nc.sync.dma_start(
                              out=et[:, 0:h, :],
                              in_=ed[b * iblk:(b + 1) * iblk, j0:j0 + h, :],
                          )
                          nc.scalar.dma_start(
                              out=et[:, h:dsz, :],
                              in_=ed[b * iblk:(b + 1) * iblk,
                                     j0 + h:j0 + dsz, :],
                          )
                      else:
                          eng = (nc.scalar if ("2" in dummy and q % 2)
                                 else nc.sync)
                          eng.dma_start(
                              out=et[:, 0:dsz, :],
                              in_=ed[b * iblk:(b + 1) * iblk, j0:j0 + dsz, :],
                          )
                      j0 += sz
                  continue
              for q, sz in enumerate(sched_b):
                  csz = sz if dummy == "dmatiny" else (16 if is_dummy else sz)
                  dsz = (16 if dummy in ("tiny", "dmatiny") else sz) \
                      if is_dummy else sz
                  et = epool.tile([128, jt, de], EDT)
                  nc.sync.dma_start(
                      out=et[:, 0:dsz, :],
                      in_=ed[b * iblk:(b + 1) * iblk, j0:j0 + dsz, :],
                  )
                  # scores_slice = sum_k E[:, :, k] * W[k]
                  stile = spool.tile([128, jt], F32, tag="stile")
                  ssl = stile[:, 0:csz]
                  if mode == "ttr":
                      # ONE fused DVE op per tile: product with broadcast W
                      # (contiguous reads) + grouped row-sum over k into the
                      # scores slice. 16x fewer DVE instructions than the
                      # scalar_tensor_tensor chain and no strided reads.
                      wbc = wall.rearrange("p k -> p () k").broadcast_to(
                          [128, csz, de])
                      nc.vector.tensor_tensor_reduce(
                          out=xscr[:, 0:csz, :], in0=et[:, 0:csz, :], in1=wbc,
                          scale=1.0, scalar=0.0, op0=ALU.mult, op1=ALU.add,
                          accum_out=stile.rearrange("p j -> p j ()")[:, 0:csz, :],
                      )
                  elif mode == "psplit":
                      # k-contraction split across three engines so no single
                      # engine's DRAIN-doubled throughput gates the DMA
                      # stream:  DVE chains k<kd; ACT forms w_k*E_k products
                      # for k>=kd which PE accumulates (identity-stationary
                      # matmuls) into a 2-bank PSUM tile; PE also folds the
                      # DVE partial in, and tanh reads straight from PSUM.
                      if kd > 0:
                          nc.vector.tensor_scalar(
                              ssl, et[:, 0:csz, 0], wall[:, 0:1], None,
                              ALU.mult)
                      for k in range(1, kd):
                          nc.vector.scalar_tensor_tensor(
                              ssl, et[:, 0:csz, k], wall[:, k:k + 1], ssl,
                              ALU.mult, ALU.add,
                          )
                      ps = qpool.tile([128, jt], F32, tag="ps")
                      mid = csz // 2 if csz <= 512 else 512
                      pieces = [(0, mid), (mid, csz)]
                      for ki, k in enumerate(range(kd, de)):
                          if pe_prod:
                              # lhsT = w_k*I -> ps += w_k * E[:, :, k]
                              for pi, (a, e) in enumerate(pieces):
                                  nc.tensor.matmul(
                                      ps[:, a:e], wident[:, k, :],
                                      et[:, a:e, k],
                                      start=(ki == 0 and (pi == 0 or a == 512)),
                                      stop=(kd == 0 and k == de - 1),
                                  )
                              continue
                          pk = prodpool.tile([128, jt], F32, tag="pk")
                          nc.scalar.activation(
                              pk[:, 0:csz], et[:, 0:csz, k], AF.Copy,
                              scale=wall_act[:, k:k + 1])
                          for pi, (a, e) in enumerate(pieces):
                              # start clears the has-written bits bank-wide;
                              # only the first touch of each bank sets it
                              nc.tensor.matmul(
                                  ps[:, a:e], ident[:], pk[:, a:e],
                                  start=(ki == 0 and (pi == 0 or a == 512)),
                                  stop=False,
                              )
                      for pi, (a, e) in enumerate(pieces):
                          if kd > 0:
                              nc.tensor.matmul(
                                  ps[:, a:e], ident[:], stile[:, a:e],
                                  start=False, stop=True,
                              )
                  elif mode == "ilv2":
                      # two independent interleaved partial chains: even k
                      # accumulate into ssl, odd k into p2sl.  Consecutive
                      # DVE instructions then never have a RAW dependency,
                      # so the engine pipeline never drains mid-chain.
                      # One extra combine add at the end.
                      p2 = i2pool.tile([128, jt], F32, tag="p2")
                      p2sl = p2[:, 0:csz]
                      nc.vector.tensor_scalar(
                          ssl, et[:, 0:csz, 0], wall[:, 0:1], None, ALU.mult)
                      nc.vector.tensor_scalar(
                          p2sl, et[:, 0:csz, 1], wall[:, 1:2], None, ALU.mult)
                      for k in range(2, dve_k):
                          dst = ssl if k % 2 == 0 else p2sl
                          nc.vector.scalar_tensor_tensor(
                              dst, et[:, 0:csz, k], wall[:, k:k + 1], dst,
                              ALU.mult, ALU.add,
                          )
                      nc.vector.tensor_tensor(ssl, p2sl, ssl, ALU.add)
                  else:
                      # chain of 16 in-place scalar_tensor_tensor ops
                      # (scalar = per-partition W[k] broadcast)
                      nc.vector.tensor_scalar(
                          ssl, et[:, 0:csz, 0], wall[:, 0:1], None, ALU.mult)
                      for k in range(1, dve_k):
                          nc.vector.scalar_tensor_tensor(
                              ssl, et[:, 0:csz, k], wall[:, k:k + 1], ssl,
                              ALU.mult, ALU.add,
                          )
                  if gp_k:
                      gtile = gpool.tile([128, jt], F32, tag="gtile")
                      gsl = gtile[:, 0:csz]
                      nc.gpsimd.tensor_scalar(
                          gsl, et[:, 0:csz, dve_k], wall_gp[:, dve_k:dve_k + 1],
                          None, ALU.mult)
                      for k in range(dve_k + 1, de):
                          nc.gpsimd.scalar_tensor_tensor(
                              gsl, et[:, 0:csz, k], wall_gp[:, k:k + 1], gsl,
                              ALU.mult, ALU.add,
                          )
                      nc.vector.tensor_tensor(ssl, gsl, ssl, ALU.add)
                  # per-slice softmax numerator + partial row-sum
                  if mode == "psplit":
                      nc.scalar.activation(ssl, ps[:, 0:csz], AF.Tanh)
                  else:
                      nc.scalar.activation(ssl, ssl, AF.Tanh)
                  nc.scalar.activation(ssl, ssl, AF.Exp,
                                       accum_out=zparts[:, q:q + 1])
                  # transpose + aggregate this slice's 128-chunks; pack_t
                  # chunks share one PSUM bank so ONE wide ACT copy stages
                  # them all (fewer per-op overheads on the ACT engine)
                  nchunk = sz // 128
                  ci = 0
                  while ci < nchunk:
                      g = min(pack_t, nchunk - ci, 4)
                      ptile = ppool.tile([128, 128 * pack_t], F32, tag="pt")
                      for u in range(g):
                          nc.tensor.transpose(
                              ptile[:, u * 128:(u + 1) * 128],
                              stile[:, (ci + u) * 128:(ci + u + 1) * 128],
                              ident[:],
                          )
                      atc = tpool.tile([128, 128 * pack_t], F32, tag="atc")
                      if copy_eng == "gpsimd":
                          # PSUM->SBUF staging on the otherwise-idle Pool
                          # engine, keeping ACT free for tanh/exp
                          nc.gpsimd.tensor_copy(
                              atc[:, 0:128 * g], ptile[:, 0:128 * g])
                      else:
                          nc.scalar.copy(
                              atc[:, 0:128 * g], ptile[:, 0:128 * g])
                      for u in range(g):
                          c = j0 // 128 + ci + u
                          nc.tensor.matmul(
                              pu[:], atc[:, u * 128:(u + 1) * 128],
                              featx[:, c, :],
                              start=(c == 0), stop=(c == nck - 1),
                          )
                      ci += g
                  j0 += sz

              zcol = mpool.tile([128, 1], F32)
              nc.vector.reduce_sum(out=zcol[:], in_=zparts[:], axis=AX.X)
              rz = mpool.tile([128, 1], F32)
              nc.vector.reciprocal(rz[:], zcol[:])

              # out = tanh(pu * rz)
              osb = opool.tile([128, df], F32)
              nc.scalar.activation(osb[:], pu[:], AF.Tanh, scale=rz[:])
              # out-DMA on the scalar-engine HWDGE ring: edge DMAs (SP ring)
              # never queue behind it (HWDGE is FIFO per issuing engine)
              nc.scalar.dma_start(
                  out=out[b * iblk:(b + 1) * iblk, :], in_=osb[:]
              )

    nc.compile()
    return nc


_CACHE = {}


def _get_nc(shape_key):
    if shape_key not in _CACHE:
        n, de, df = shape_key
        _CACHE[shape_key] = build(n=n, de=de, df=df, edt=EDGE_DT)
    return _CACHE[shape_key]


def build_default(**kw):
    """build() with the shipping edge dtype (for test harness use)."""
    return build(edt=EDGE_DT, **kw)


def kernel(edges, features, W):
    n, n2, de = edges.shape
    df = features.shape[1]
    rows = n // N_CORES
    nc = _get_nc((n, de, df))

    # bf16 edge stream halves the HBM traffic of the one tensor that
    # dominates it (1.07GB fp32 -> 537MB); all arithmetic stays fp32
    if EDGE_DT == "bf16":
        edges = np.ascontiguousarray(edges).astype(mybir.dt.np(BF16))
    else:
        edges = np.ascontiguousarray(edges, dtype=np.float32)
    features = np.ascontiguousarray(features, dtype=np.float32)
    W = np.ascontiguousarray(W, dtype=np.float32)

    in_maps = [
        {
            "edges": edges[c * rows:(c + 1) * rows],
            "features": features,
            "W": W,
        }
        for c in range(N_CORES)
    ]
    res = run_bass_kernel_spmd(nc, in_maps, core_ids=list(range(N_CORES)))
    return np.concatenate([r["out"] for r in res.results], axis=0)

